# revision 19
# baseline (speedup 1.0000x reference)
"""Trainium2 Bass kernel for nn_Attention_43963285242601.

GQA attention block: q/k/v projections + RoPE + causal attention + o_proj,
tensor-parallel over 8 NeuronCores.

Sharding (core c of 8):
  - q-heads 4c..4c+3 and kv-head c: Wq/Wk/Wv column (head) shards,
    attention fully local per head group.
  - o_proj sharded over Wo ROWS (output features): every core computes
    out[:, 512c:512c+512] and needs the full attention output, which is
    distributed via four AllGathers (batch x head-pair, bf16).
  - host concatenates the 8 feature shards: no all-reduce needed.

v2: single fused emission stream.  Projections for each 512-token group
(2 passes over x: [k,v,q0] then [q1,q2,q3] to keep only 3 proj PSUM banks
live), RoPE straight out of PSUM on DVE, and attention for a token group
is emitted as micro-units interleaved between later projection matmuls so
the PE never stalls on softmax (ACT) latency.  Causal mask is applied as
a post-exp bf16 multiply on DVE (removes mask matmuls from PE), softmax
P / acc run in bf16, the denominator comes from a GpSimd partition-reduce
and is broadcast via a stride-0 AP (removes sum/broadcast matmuls).
AllGathers for batch 0 fire mid-kernel; o_proj(b0) hides the batch-1
AllGathers; o_proj slabs contract head-pair-0 blocks first so the last
AllGather is fully hidden.
"""

import numpy as np

import concourse.bacc as bacc
import concourse.bass_isa as bass_isa
import concourse.mybir as mybir
import concourse.tile as tile
from concourse.bass_utils import run_bass_kernel_spmd

F32 = mybir.dt.float32
F32R = mybir.dt.float32r
BF16 = mybir.dt.bfloat16
AF = mybir.ActivationFunctionType

N_CORES = 8
B, L = 2, 2048
N_HEADS, N_KV = 32, 8
HEAD_DIM = 128
D = N_HEADS * HEAD_DIM
THETA = 500000.0

EXP_BIAS = -8.0


def _rope_tables(t_all, l, dh):
    half = dh // 2
    inv = 1.0 / (THETA ** (np.arange(half, dtype=np.float64) * 2.0 / dh))
    pos = np.arange(t_all, dtype=np.float64) % l
    ang = inv[:, None] * pos[None, :]  # [half, T]
    cos = np.cos(ang)
    sin = np.sin(ang)
    return (
        np.concatenate([cos, cos], 0).astype(np.float32),
        np.concatenate([sin, sin], 0).astype(np.float32),
    )


def _build(n_cores=N_CORES, b=B, l=L, nh=N_HEADS, nkv=N_KV):
    import ml_dtypes

    dh = HEAD_DIM
    d = nh * dh
    t_all = b * l
    hpc = nh // n_cores  # q heads per core
    assert nkv == n_cores, "one kv head per core"
    mpc = d // n_cores  # o_proj output features per core
    kt_d = d // dh  # contraction tiles for projections
    ktl = l // 128  # key tiles per batch
    tg_n = t_all // 512  # 512-wide token groups
    ksub = 4  # k-tiles per x subslab load
    nsub = kt_d // ksub
    n_hp = hpc // 2  # head-pairs per core
    scale = dh ** -0.5

    nc = bacc.Bacc(
        "TRN2", target_bir_lowering=False, debug=False, num_devices=n_cores
    )

    xT = nc.dram_tensor("xT", [d, t_all], F32R, kind="ExternalInput").ap()
    wqT = nc.dram_tensor("wqT", [d, hpc * dh], F32R, kind="ExternalInput").ap()
    wkT = nc.dram_tensor("wkT", [d, dh], F32R, kind="ExternalInput").ap()
    wvT = nc.dram_tensor("wvT", [d, dh], F32R, kind="ExternalInput").ap()
    woT = nc.dram_tensor("woT", [d, mpc], BF16, kind="ExternalInput").ap()
    outT = nc.dram_tensor("outT", [mpc, t_all], F32, kind="ExternalOutput").ap()

    # compile-time constants
    cos_np, sin_np = _rope_tables(t_all, l, dh)
    # multiplicative bf16 causal mask for the 4 diagonal key tiles of each
    # 512-query group: maskP[k, j*512 + q] = (128*j + k <= q)
    k_idx = np.arange(128)[:, None]
    q_idx = np.arange(512)[None, :]
    maskp_np = np.zeros((128, 4 * 512), dtype=ml_dtypes.bfloat16)
    for j in range(4):
        maskp_np[:, j * 512 : (j + 1) * 512] = (128 * j + k_idx <= q_idx).astype(
            ml_dtypes.bfloat16
        )
    cos_c = nc.inline_tensor(cos_np, name="cos_c").ap()
    sin_c = nc.inline_tensor(sin_np, name="sin_c").ap()
    maskp_c = nc.inline_tensor(maskp_np, name="maskp_c").ap()
    ident_c = nc.inline_tensor(np.eye(128, dtype=np.float32), name="ident_c").ap()

    with tile.TileContext(nc) as tc:
        with (
            tc.tile_pool(name="constp", bufs=1) as constp,
            tc.tile_pool(name="kvp", bufs=1) as kvp,
            tc.tile_pool(name="qstp", bufs=8) as qstp,
            tc.tile_pool(name="cspool", bufs=2) as cspool,
            tc.tile_pool(name="ropet", bufs=3) as ropet,
            tc.tile_pool(name="vtst", bufs=2) as vtst,
            tc.tile_pool(name="ppool", bufs=3) as ppool,
            tc.tile_pool(name="accp", bufs=2) as accp,
            tc.tile_pool(name="rsb", bufs=2) as rsb,
            tc.tile_pool(name="obf", bufs=2) as obf,
            tc.tile_pool(name="dramp", bufs=1, space="DRAM") as dramp,
            tc.tile_pool(name="pspp", bufs=1, space="PSUM") as pspp,
            tc.tile_pool(name="pop", bufs=1, space="PSUM") as pop,
        ):
            masksb = constp.tile([128, 4 * 512], BF16, tag="masksb")
            nc.gpsimd.dma_start(masksb[:], maskp_c)
            ident = constp.tile([128, 128], F32, tag="ident")
            nc.gpsimd.dma_start(ident[:], ident_c)
            bias_t = constp.tile([128, 1], F32, tag="bias_t")
            nc.vector.memset(bias_t[:], EXP_BIAS)

            # per-batch K (rotated, [dh, l]) and Vn ([key, l-of-dh-cols]) buffers
            K2 = [kvp.tile([128, l], F32R, tag=f"K{i}", name=f"K{i}") for i in range(2)]
            Vn2 = [
                kvp.tile([128, l], BF16, tag=f"Vn{i}", name=f"Vn{i}")
                for i in range(2)
            ]

            bounce = [
                [
                    dramp.tile([2 * dh, l], BF16, tag=f"bounce{bb}_{hp}",
                               name=f"bounce{bb}_{hp}")
                    for hp in range(n_hp)
                ]
                for bb in range(b)
            ]
            gathered = [
                [
                    dramp.tile(
                        [n_cores * 2 * dh, l], BF16,
                        addr_space="Shared" if n_cores > 4 else "Local",
                        tag=f"gath{bb}_{hp}", name=f"gath{bb}_{hp}"
                    )
                    for hp in range(n_hp)
                ]
                for bb in range(b)
            ]

            wq_r = wqT.rearrange("(k p) m -> p k m", p=128)
            wk_r = wkT.rearrange("(k p) m -> p k m", p=128)
            wv_r = wvT.rearrange("(k p) m -> p k m", p=128)
            xT_r = xT.rearrange("(k p) t -> p k t", p=128)

            # ---- attention micro-unit machinery --------------------------
            # pending: FIFO of (batch, closure).  Units are popped between
            # projection matmul blocks so softmax latency hides under PE
            # work that is independent of it.
            pending = []

            def drain(k):
                for _ in range(min(k, len(pending))):
                    pending.pop(0)[1]()

            def drain_all():
                while pending:
                    pending.pop(0)[1]()

            def drain_batch(bb):
                while pending and pending[0][0] == bb:
                    pending.pop(0)[1]()

            def make_group_units(bb, h, g, qt):
                """Attention for (batch bb, local head h, 512-query group g).
                qt: rotated q tile [128, 512] f32r.  Appends units to pending."""
                nkt = 4 * g + 4
                K = K2[bb]
                Vn = Vn2[bb]
                po = {}
                acc = {}
                state = {}

                def mk_scores(kt):
                    def u():
                        psp = pspp.tile([128, 512], F32, tag="psp", name="psp")
                        state[("psp", kt)] = psp
                        nc.tensor.matmul(
                            psp[:],
                            K[:, kt * 128 : (kt + 1) * 128],
                            qt[:],
                            start=True,
                            stop=True,
                            skip_group_check=True,
                        )
                    return u

                def mk_softpv(kt):
                    def u():
                        psp = state.pop(("psp", kt))
                        P = ppool.tile([128, 512], BF16, tag="P", name="P")
                        nc.scalar.activation(
                            P[:], psp[:], AF.Exp, scale=scale, bias=bias_t[:]
                        )
                        j = kt - 4 * g  # index into diagonal-mask blocks
                        if j >= 0:
                            nc.vector.tensor_mul(
                                P[:], P[:], masksb[:, j * 512 : (j + 1) * 512]
                            )
                        if kt == 0:
                            po["t"] = pop.tile([128, 512], F32, tag="po",
                                               name="po")
                            acc["t"] = accp.tile([128, 512], BF16, tag="acc",
                                                 name="acc")
                        nc.tensor.matmul(
                            po["t"][:],
                            Vn[:, kt * 128 : (kt + 1) * 128],
                            P[:],
                            start=(kt == 0),
                            stop=(kt == nkt - 1),
                            skip_group_check=True,
                        )
                        if kt == 0:
                            nc.vector.tensor_copy(acc["t"][:], P[:])
                        else:
                            nc.vector.tensor_add(acc["t"][:], acc["t"][:], P[:])
                        if kt == nkt - 1:
                            # evacuate po so its bank frees before the slow
                            # softmax tail; the tail then runs off SBUF
                            po["sb"] = obf.tile([128, 512], F32, tag="posb",
                                                name="posb")
                            nc.scalar.activation(po["sb"][:], po["t"][:],
                                                 AF.Copy)
                    return u

                def tail():
                    # denominator: every partition gets sum-over-keys (Q7
                    # daisy-chain), so no broadcast matmul is needed
                    bs = rsb.tile([128, 512], F32, tag="bs", name="bs")
                    nc.gpsimd.partition_all_reduce(
                        bs[:], acc["t"][:], 128, bass_isa.ReduceOp.add
                    )
                    bsr = rsb.tile([128, 512], F32, tag="bsr", name="bsr")
                    nc.vector.reciprocal(bsr[:], bs[:])
                    ob = obf.tile([128, 512], BF16, tag="ob", name="ob")
                    nc.vector.tensor_mul(ob[:], po["sb"][:], bsr[:])
                    nc.sync.dma_start(
                        bounce[bb][h // 2][
                            (h % 2) * dh : (h % 2 + 1) * dh,
                            g * 512 : (g + 1) * 512,
                        ],
                        ob[:],
                    )

                # scores run one key-tile ahead of softmax+PV so exp latency
                # is always covered by in-flight PE work
                units = [mk_scores(0)]
                for kt in range(1, nkt):
                    units.append(mk_scores(kt))
                    units.append(mk_softpv(kt - 1))
                units.append(mk_softpv(nkt - 1))
                units.append(tail)
                pending.extend((bb, u) for u in units)

            # ---- weights (persistent) -----------------------------------
            with (
                tc.tile_pool(name="wpool", bufs=1) as wpool,
                tc.tile_pool(name="xpool", bufs=2) as xpool,
                tc.tile_pool(name="psproj", bufs=1, space="PSUM") as psproj,
            ):
                wq_sb = wpool.tile([128, kt_d, hpc * dh], F32R, tag="wq")
                wk_sb = wpool.tile([128, kt_d, dh], F32R, tag="wk")
                wv_sb = wpool.tile([128, kt_d, dh], F32R, tag="wv")

                def emit_tg(tg):
                    bb = tg // 4
                    g = tg % 4
                    toff = tg * 512
                    tloc = g * 512

                    cos_sb = cspool.tile([128, 512], F32, tag="cos")
                    nc.gpsimd.dma_start(cos_sb[:], cos_c[:, toff : toff + 512])
                    sin_sb = cspool.tile([128, 512], F32, tag="sin")
                    nc.gpsimd.dma_start(sin_sb[:], sin_c[:, toff : toff + 512])

                    def rope(dst_ap, src_ap):
                        # dst[0:64]  = s[0:64]*cos - s[64:]*sin
                        # dst[64:]   = s[64:]*cos + s[0:64]*sin
                        tc_ = ropet.tile([128, 512], F32, tag="rtc", name="rtc")
                        ts_ = ropet.tile([128, 512], F32, tag="rts", name="rts")
                        nc.vector.tensor_mul(tc_[:], src_ap, cos_sb[:])
                        # swapped-half sin products (out base differs from in)
                        nc.vector.tensor_mul(
                            ts_[0:64, :], src_ap[64:128, :], sin_sb[64:128, :]
                        )
                        nc.vector.tensor_mul(
                            ts_[64:128, :], src_ap[0:64, :], sin_sb[0:64, :]
                        )
                        nc.vector.tensor_sub(
                            dst_ap[0:64, :], tc_[0:64, :], ts_[0:64, :]
                        )
                        nc.vector.tensor_add(
                            dst_ap[64:128, :], tc_[64:128, :], ts_[64:128, :]
                        )

                    # ---------- single pass over x: k, v, q0..q3 ----------
                    pk = psproj.tile([128, 512], F32, tag="pa", name="pa")
                    pv = psproj.tile([128, 512], F32, tag="pb", name="pb")
                    pq = [
                        psproj.tile([128, 512], F32, tag=t, name=t)
                        for t in ("pc", "pd", "pe", "pf")
                    ]
                    blocks = [(pk, wk_sb, 0), (pv, wv_sb, 0)] + [
                        (pq[o], wq_sb, o * dh) for o in range(hpc)
                    ]
                    for sub in range(nsub):
                        ks = slice(sub * ksub, (sub + 1) * ksub)
                        if tg == 0:
                            nc.gpsimd.dma_start(wq_sb[:, ks, :], wq_r[:, ks, :])
                            nc.gpsimd.dma_start(wk_sb[:, ks, :], wk_r[:, ks, :])
                            nc.gpsimd.dma_start(wv_sb[:, ks, :], wv_r[:, ks, :])
                        xs = xpool.tile([128, ksub, 512], F32R, tag="xs")
                        nc.sync.dma_start(xs[:], xT_r[:, ks, toff : toff + 512])
                        for dst, w_sb, o0 in blocks:
                            for k in range(ksub):
                                kt = sub * ksub + k
                                nc.tensor.matmul(
                                    dst[:], w_sb[:, kt, o0 : o0 + dh],
                                    xs[:, k, :],
                                    start=(kt == 0), stop=(kt == kt_d - 1),
                                )
                            drain(2)

                    # k: rope from PSUM into K2
                    rope(K2[bb][:, tloc : tloc + 512], pk[:])
                    # v: copy staging, transpose 128-blocks on PE, to Vn
                    vt = vtst.tile([128, 512], F32, tag="vt", name="vt")
                    nc.scalar.activation(vt[:], pv[:], AF.Copy)
                    pt = pspp.tile([128, 512], F32, tag="psp", name="psp")
                    for j in range(4):
                        nc.tensor.transpose(
                            pt[:, j * 128 : (j + 1) * 128],
                            vt[:, j * 128 : (j + 1) * 128],
                            ident[:],
                        )
                    nc.vector.tensor_copy(
                        Vn2[bb][:, tloc : tloc + 512],
                        pt[:],
                    )
                    # q ropes + enqueue this token group's attention
                    qts = [qstp.tile([128, 512], F32R, tag="qst", name="qst")
                           for _ in range(hpc)]
                    for o in range(hpc):
                        rope(qts[o][:], pq[o][:])
                        make_group_units(bb, o, g, qts[o])
                        drain(2)

                for tg in range(tg_n):
                    emit_tg(tg)
                    if tg == 5:
                        # all batch-0 groups are enqueued by tg3; force any
                        # stragglers out so the bounce writes precede the
                        # collectives in emission order.
                        drain_batch(0)
                        for hp in range(n_hp):
                            nc.gpsimd.collective_compute(
                                "AllGather",
                                mybir.AluOpType.bypass,
                                replica_groups=[list(range(n_cores))],
                                ins=[bounce[0][hp].opt()],
                                outs=[gathered[0][hp].opt()],
                            )

            # wq/wk/wv/xs pools and proj PSUM released here.
            with (
                tc.tile_pool(name="wopool", bufs=1) as wopool,
                tc.tile_pool(name="ogpool", bufs=2) as ogpool,
                tc.tile_pool(name="outst", bufs=3) as outst,
                tc.tile_pool(name="pso", bufs=2, space="PSUM") as pso,
            ):
                wo_sb = wopool.tile([128, kt_d, mpc], BF16, tag="wo")
                nc.gpsimd.dma_start(
                    wo_sb[:], woT.rearrange("(k p) m -> p k m", p=128)
                )

                # og block j (j = hp * (n_cores*2) + c*2 + hl) holds global
                # head 4c + 2hp + hl; contract the matching wo column.
                kt_map = []
                for hp in range(n_hp):
                    for c in range(n_cores):
                        for hl in range(2):
                            kt_map.append(4 * c + 2 * hp + hl)
                blk = n_cores * 2

                def oproj_slab(bb, tgl):
                    g_rs = [
                        gathered[bb][hp][:].rearrange("(k p) t -> p k t", p=128)
                        for hp in range(n_hp)
                    ]
                    og = ogpool.tile([128, kt_d, 512], BF16, tag="og", name="og")
                    for hp in range(n_hp):
                        nc.gpsimd.dma_start(
                            og[:, hp * blk : (hp + 1) * blk, :],
                            g_rs[hp][:, :, tgl * 512 : (tgl + 1) * 512],
                        )
                    # keep leftover attention units ahead of matmuls that
                    # wait on the og/wo DMAs
                    drain(2)
                    for m in range(mpc // 128):
                        pp = pso.tile([128, 512], F32, tag="pp", name="pp")
                        for kt in range(kt_d):
                            nc.tensor.matmul(
                                pp[:],
                                wo_sb[:, kt_map[kt], m * 128 : (m + 1) * 128],
                                og[:, kt, :],
                                start=(kt == 0),
                                stop=(kt == kt_d - 1),
                            )
                            if kt % 4 == 3:
                                drain(1)
                        ot = outst.tile([128, 512], F32, tag="ot", name="ot")
                        nc.scalar.activation(ot[:], pp[:], AF.Copy)
                        nc.sync.dma_start(
                            outT[
                                m * 128 : (m + 1) * 128,
                                bb * l + tgl * 512 : bb * l + (tgl + 1) * 512,
                            ],
                            ot[:],
                        )

                # batch-0 o_proj interleaved with tg7's leftover attention;
                # pre-drain a few units so the PE has work while wo/og load
                drain(6)
                for tgl in range(l // 512):
                    oproj_slab(0, tgl)
                drain_all()
                for hp in range(n_hp):
                    nc.gpsimd.collective_compute(
                        "AllGather",
                        mybir.AluOpType.bypass,
                        replica_groups=[list(range(n_cores))],
                        ins=[bounce[1][hp].opt()],
                        outs=[gathered[1][hp].opt()],
                    )
                for tgl in range(l // 512):
                    oproj_slab(1, tgl)

    nc.compile()
    return nc


_NC_CACHE = {}


def _get_nc(key=(N_CORES, B, L, N_HEADS, N_KV)):
    if key not in _NC_CACHE:
        _NC_CACHE[key] = _build(*key)
    return _NC_CACHE[key]


def make_in_maps(x, Wq, Wk, Wv, Wo, n_cores=N_CORES):
    import ml_dtypes

    b, l, d = x.shape
    nh = Wq.shape[0] // HEAD_DIM
    hpc = nh // n_cores
    mpc = d // n_cores
    xT = np.ascontiguousarray(x.reshape(b * l, d).T.astype(np.float32))
    in_maps = []
    for c in range(n_cores):
        wq_c = np.ascontiguousarray(
            Wq[c * hpc * HEAD_DIM : (c + 1) * hpc * HEAD_DIM, :].T.astype(np.float32)
        )
        wk_c = np.ascontiguousarray(
            Wk[c * HEAD_DIM : (c + 1) * HEAD_DIM, :].T.astype(np.float32)
        )
        wv_c = np.ascontiguousarray(
            Wv[c * HEAD_DIM : (c + 1) * HEAD_DIM, :].T.astype(np.float32)
        )
        wo_c = np.ascontiguousarray(
            Wo[c * mpc : (c + 1) * mpc, :].T.astype(ml_dtypes.bfloat16)
        )
        in_maps.append(
            {"xT": xT, "wqT": wq_c, "wkT": wk_c, "wvT": wv_c, "woT": wo_c}
        )
    return in_maps


def assemble_out(results, b, l, d):
    parts = [r["outT"] for r in results]
    outT = np.concatenate(parts, axis=0)  # [D, T]
    return np.ascontiguousarray(outT.T).reshape(b, l, d).astype(np.float32)


def kernel(x, Wq, Wk, Wv, Wo, trace=False, tmpdir=None):
    x = np.asarray(x, dtype=np.float32)
    nc = _get_nc()
    in_maps = make_in_maps(x, Wq, Wk, Wv, Wo)
    res = run_bass_kernel_spmd(
        nc, in_maps, list(range(N_CORES)), trace=trace, tmpdir=tmpdir
    )
    out = assemble_out(res.results, *x.shape)
    if trace:
        return out, res
    return out


if __name__ == "__main__":
    rng = np.random.default_rng(0)
    s = 0.02
    x = rng.standard_normal((B, L, D)).astype(np.float32)
    Wq = (rng.standard_normal((D, D)) * s).astype(np.float32)
    Wk = (rng.standard_normal((N_KV * HEAD_DIM, D)) * s).astype(np.float32)
    Wv = (rng.standard_normal((N_KV * HEAD_DIM, D)) * s).astype(np.float32)
    Wo = (rng.standard_normal((D, D)) * s).astype(np.float32)
    out = kernel(x, Wq, Wk, Wv, Wo)
    print(out.shape, out.dtype)
